# revision 13
# baseline (speedup 1.0000x reference)
"""Trainium2 Bass kernel for nn_DTPByHead (gnn_message_passing).

Reference computation (per head h, edge z, channel m, k in 0..2):
    w4 = weights.reshape(E, 4, H, M) -> w_ss, w_sv, w_vs, w_vv
    s_out   = w_s @ [w_ss*s ; w_vv*(v.r)] + b_s
    v_out_k = w_v @ [w_sv*s*r_k ; w_vs*v_k]

Kernel strategy (edges data-parallel over 8 cores; fp16 compute, fp32 out):
  Per 128-edge block (edges on partitions):
    pieces a=w_ss*s, d=w_vv*(v.r), b_k=(w_sv*s)*r_k, c_k=w_vs*v_k
    X1 = [a|d|b2|c2], X2 = [b0|c0|b1|c1]  (128 cols each, block-major)
    PE transpose X1,X2 -> PSUM -> ACT copy to SBUF
    PE matmul: out1 = X1T.T @ W1 = [s_out|v2], out2 = X2T.T @ W2 = [v0|v1]
  W1/W2 are host-prepacked block matrices of w_s/w_v so outputs come out
  edge-major (no back-transpose). Inputs are host-rearranged into the exact
  per-partition SBUF layouts (v deinterleaved k-major) so every DMA is
  contiguous and every elementwise op runs in the DVE 2x fp16 mode.
"""

import numpy as np

H = 8
E = 65536
M = 32
NCORES = 8
EL = E // NCORES  # 8192 edges per core
G = 8             # 128-edge blocks per supertile
SUP = EL // (128 * G)

_CACHE = {}

# scheduling/buffering knobs (model-tuned)
TUNE = {
    "inp_bufs": 2,
    "outp_bufs": 2,
    "xp_bufs": 2,
    "xtp_bufs": 2,
    "tmp_bufs": 2,
    "tps_bufs": 2,
    "ops_bufs": 2,
}


def _build_module(el, g, loop_reps=0):
    """Build + compile the per-core Bass module. g = 128-edge blocks per supertile."""
    from contextlib import ExitStack

    import concourse.bacc as bacc
    import concourse.tile as tile
    from concourse import mybir

    f16 = mybir.dt.float16
    f32 = mybir.dt.float32
    Copy = mybir.ActivationFunctionType.Copy

    blk = 128
    sup = el // (blk * g)
    assert sup * blk * g == el
    GM = g * M          # free size of one piece over a supertile

    nc = bacc.Bacc(
        "TRN2",
        target_bir_lowering=False,
        debug=False,
        enable_asserts=False,
        num_devices=NCORES,
    )

    # Host-prearranged layouts (see _prep_maps): everything partition-major.
    s_d = nc.dram_tensor("s16", [sup, 128, H * GM], f16, kind="ExternalInput").ap()
    v_d = nc.dram_tensor("v16", [sup, 128, H * 3 * GM], f16, kind="ExternalInput").ap()
    w_d = nc.dram_tensor("w16", [sup, 128, g * 1024], f16, kind="ExternalInput").ap()
    r_d = nc.dram_tensor("r32", [sup, 128, g * 3], f32, kind="ExternalInput").ap()
    wp_d = nc.dram_tensor("wpack", [H, 128, 128], f16, kind="ExternalInput").ap()
    bb_d = nc.dram_tensor("bconst", [128, H * M], f32, kind="ExternalInput").ap()
    id_d = nc.dram_tensor("ident", [128, 128], f16, kind="ExternalInput").ap()
    so_d = nc.dram_tensor("so", [sup, 128, H * GM], f32, kind="ExternalOutput").ap()
    vo_d = nc.dram_tensor("vo", [sup, 128, H * 3 * GM], f32, kind="ExternalOutput").ap()

    with tile.TileContext(nc) as tc, ExitStack() as ctx:
        const = ctx.enter_context(tc.tile_pool(name="const", bufs=1))
        inp = ctx.enter_context(tc.tile_pool(name="inp", bufs=TUNE["inp_bufs"]))
        xp = ctx.enter_context(tc.tile_pool(name="xp", bufs=TUNE["xp_bufs"]))
        xtp = ctx.enter_context(tc.tile_pool(name="xtp", bufs=TUNE["xtp_bufs"]))
        tmp = ctx.enter_context(tc.tile_pool(name="tmp", bufs=TUNE["tmp_bufs"]))
        outp = ctx.enter_context(tc.tile_pool(name="outp", bufs=TUNE["outp_bufs"]))
        tps = ctx.enter_context(tc.tile_pool(name="tps", bufs=TUNE["tps_bufs"], space="PSUM"))
        ops = ctx.enter_context(tc.tile_pool(name="ops", bufs=TUNE["ops_bufs"], space="PSUM"))

        wp = const.tile([128, H * 128], f16)
        nc.sync.dma_start(
            wp[:].rearrange("p (h c) -> p h c", c=128),
            wp_d.rearrange("h p c -> p h c"),
        )
        bb = const.tile([128, H * M], f32)
        nc.sync.dma_start(bb[:], bb_d)
        idn = const.tile([128, 128], f16)
        nc.sync.dma_start(idn[:], id_d)

        if loop_reps:
            engines = [
                mybir.EngineType.PE,
                mybir.EngineType.DVE,
                mybir.EngineType.Activation,
                mybir.EngineType.SP,
                mybir.EngineType.Pool,
            ]
            ctx.enter_context(tc.For_i(0, loop_reps, 1, hint_engines=engines))

        for t in range(sup):
            s_all = inp.tile([128, H * GM], f16, tag="s")
            nc.sync.dma_start(s_all[:], s_d[t])
            v_all = inp.tile([128, H * 3 * GM], f16, tag="v")
            nc.sync.dma_start(v_all[:], v_d[t])
            w4 = inp.tile([128, g * 1024], f16, tag="w4")
            nc.sync.dma_start(w4[:], w_d[t])
            rt = inp.tile([128, g * 3], f32, tag="r")
            nc.sync.dma_start(rt[:], r_d[t])

            # rb[:, k*GM + g_*M + m] = r[edge(p,g_), k]  (fp16, broadcast over m)
            rb = tmp.tile([128, 3 * GM], f16, tag="rb")
            rt3 = rt[:].rearrange("p (g k) -> p g k", k=3)
            for k in range(3):
                nc.vector.tensor_copy(
                    rb[:, k * GM : (k + 1) * GM].rearrange("p (g m) -> p g m", m=M),
                    rt3[:, :, k : k + 1].broadcast_to((128, g, M)),
                )

            so_all = outp.tile([128, H * GM], f32, tag="so")
            vo_all = outp.tile([128, H * 3 * GM], f32, tag="vo")

            for h0 in range(0, H, 2):
                # ---- pair-batched elementwise (DVE + Pool): dims (p, h2, g, m) ----
                X1 = xp.tile([128, 2 * g * 128], f16, tag="x1")  # [a | d | b2 | c2]
                X2 = xp.tile([128, 2 * g * 128], f16, tag="x2")  # [b0 | c0 | b1 | c1]

                def sH2():
                    # s cols (h, g, m)
                    return s_all[:].rearrange("p (h g m) -> p h g m", h=H, m=M)[
                        :, h0 : h0 + 2
                    ]

                def wsl2(c):
                    # w4 cols (g, c256): (h2, g, m) slice at col c*256 + h*32
                    return (
                        w4[:]
                        .rearrange("p (g hc) -> p g hc", hc=1024)[
                            :, :, c * 256 + h0 * M : c * 256 + (h0 + 2) * M
                        ]
                        .rearrange("p g (h m) -> p h g m", m=M)
                    )

                def vsl2(k):
                    # v cols (h, k, g, m)
                    return v_all[:].rearrange(
                        "p (h k g m) -> p h k g m", h=H, k=3, m=M
                    )[:, h0 : h0 + 2, k]

                def rbk2(k):
                    return (
                        rb[:, k * GM : (k + 1) * GM]
                        .rearrange("p (g m) -> p g m", m=M)
                        .unsqueeze(1)
                        .broadcast_to((128, 2, g, M))
                    )

                def xsl2(X, piece):
                    # X cols (h2, g, c): block-major per head-half
                    return X[:].rearrange("p (h gc) -> p h gc", h=2)[
                        :, :, :
                    ].rearrange("p h (g c) -> p h g c", c=128)[
                        :, :, :, piece * M : (piece + 1) * M
                    ]

                bt = tmp.tile([128, 2 * GM], f16, tag="bt")
                m0 = tmp.tile([128, 2 * GM], f16, tag="m0")
                m1 = tmp.tile([128, 2 * GM], f16, tag="m1")
                m2 = tmp.tile([128, 2 * GM], f16, tag="m2")
                u0 = tmp.tile([128, 2 * GM], f16, tag="u0")
                u1 = tmp.tile([128, 2 * GM], f16, tag="u1")

                def t4(tile2d):
                    return tile2d[:].rearrange("p (h g m) -> p h g m", h=2, m=M)

                nc.vector.tensor_mul(xsl2(X1, 0), sH2(), wsl2(0))      # a
                nc.vector.tensor_mul(t4(bt), sH2(), wsl2(1))           # b
                nc.vector.tensor_mul(t4(m0), vsl2(0), rbk2(0))
                nc.vector.tensor_mul(t4(m1), vsl2(1), rbk2(1))
                nc.vector.tensor_mul(t4(m2), vsl2(2), rbk2(2))
                nc.vector.tensor_add(u0[:], m0[:], m1[:])
                nc.vector.tensor_add(u1[:], u0[:], m2[:])
                nc.vector.tensor_mul(xsl2(X1, 1), t4(u1), wsl2(3))     # d
                nc.vector.tensor_mul(xsl2(X2, 0), t4(bt), rbk2(0))     # b0
                nc.vector.tensor_mul(xsl2(X2, 2), t4(bt), rbk2(1))     # b1
                nc.vector.tensor_mul(xsl2(X1, 2), t4(bt), rbk2(2))     # b2
                nc.gpsimd.tensor_mul(xsl2(X2, 1), vsl2(0), wsl2(2))    # c0
                nc.gpsimd.tensor_mul(xsl2(X2, 3), vsl2(1), wsl2(2))    # c1
                nc.gpsimd.tensor_mul(xsl2(X1, 3), vsl2(2), wsl2(2))    # c2

                for h2 in range(2):
                    h = h0 + h2
                    xoff = h2 * g * 128

                    # ---- PE transposes ----
                    tt = tps.tile([128, g * 256], f16, tag="tps")
                    for gi in range(g):
                        for xi, X in enumerate((X1, X2)):
                            nc.tensor.transpose(
                                tt[:, gi * 256 + xi * 128 : gi * 256 + (xi + 1) * 128],
                                X[:, xoff + gi * 128 : xoff + (gi + 1) * 128],
                                idn[:],
                            )

                    # PSUM -> SBUF copy of transposed blocks (ACT 3/4 + DVE 1/4)
                    xt = xtp.tile([128, g * 256], f16, tag="xt")
                    q = g * 256 // 4
                    nc.scalar.activation(xt[:, 0 : 3 * q], tt[:, 0 : 3 * q], Copy)
                    nc.vector.tensor_copy(xt[:, 3 * q :], tt[:, 3 * q :])

                    # ---- PE matmuls ----
                    op = ops.tile([128, g * 128], f32, tag="ops")
                    for gi in range(g):
                        nc.tensor.matmul(
                            op[:, gi * 128 : gi * 128 + 64],
                            xt[:, gi * 256 : gi * 256 + 128],
                            wp[:, h * 128 : h * 128 + 64],
                        )
                        nc.tensor.matmul(
                            op[:, gi * 128 + 64 : (gi + 1) * 128],
                            xt[:, gi * 256 + 128 : (gi + 1) * 256],
                            wp[:, h * 128 + 64 : (h + 1) * 128],
                        )

                    # ---- out copies (batched over g) ----
                    def g3(ap2d):
                        return ap2d.rearrange("p (g m) -> p g m", m=M)

                    op3 = op[:].rearrange("p (g c) -> p g c", c=128)
                    nc.vector.tensor_add(
                        g3(so_all[:, h * GM : (h + 1) * GM]),
                        op3[:, :, 0:M],
                        bb[:, h * M : (h + 1) * M]
                        .unsqueeze(1)
                        .broadcast_to((128, g, M)),
                    )
                    voH = vo_all[:, h * 3 * GM : (h + 1) * 3 * GM].rearrange(
                        "p (g m k) -> p g k m", m=M, k=3
                    )
                    nc.scalar.activation(voH[:, :, 2, :], op3[:, :, M : 2 * M], Copy)
                    nc.scalar.activation(
                        voH[:, :, 0:2, :],
                        op3[:, :, 2 * M : 4 * M].rearrange(
                            "p g (k m) -> p g k m", m=M
                        ),
                        Copy,
                    )

                # per-pair stores overlap later pairs' compute
                nc.sync.dma_start(
                    so_d[t][:, h0 * GM : (h0 + 2) * GM],
                    so_all[:, h0 * GM : (h0 + 2) * GM],
                )
                nc.sync.dma_start(
                    vo_d[t][:, h0 * 3 * GM : (h0 + 2) * 3 * GM],
                    vo_all[:, h0 * 3 * GM : (h0 + 2) * 3 * GM],
                )



    nc.compile()
    return nc


def _prep_maps(inputs, el, g=G):
    """Host-side prep: cast fp16, rearrange into per-core partition-major layouts."""
    s = np.asarray(inputs["s"])
    v = np.asarray(inputs["v"])
    r = np.asarray(inputs["r_ij_vec"], dtype=np.float32)
    w = np.asarray(inputs["weights"])
    w_s = np.asarray(inputs["w_s"], dtype=np.float32)
    b_s = np.asarray(inputs["b_s"], dtype=np.float32)
    w_v = np.asarray(inputs["w_v"], dtype=np.float32)

    h, e, m = s.shape
    sup = el // (128 * g)
    ncores = e // el

    wpack = np.zeros((h, 128, 128), np.float32)
    for hh in range(h):
        # W1 rows [a; d; b2; c2] -> cols [s_out | v2]
        wpack[hh, 0:32, 0:32] = w_s[hh][:, 0:32].T
        wpack[hh, 32:64, 0:32] = w_s[hh][:, 32:64].T
        wpack[hh, 64:96, 32:64] = w_v[hh][:, 0:32].T
        wpack[hh, 96:128, 32:64] = w_v[hh][:, 32:64].T
        # W2 rows [b0; c0; b1; c1] -> cols [v0 | v1]
        wpack[hh, 0:32, 64:96] = w_v[hh][:, 0:32].T
        wpack[hh, 32:64, 64:96] = w_v[hh][:, 32:64].T
        wpack[hh, 64:96, 96:128] = w_v[hh][:, 0:32].T
        wpack[hh, 96:128, 96:128] = w_v[hh][:, 32:64].T
    wpack = wpack.astype(np.float16)
    bconst = np.ascontiguousarray(
        np.broadcast_to(b_s.reshape(1, h * m), (128, h * m)), dtype=np.float32
    )
    ident = np.eye(128, dtype=np.float16)

    # Global rearranges (edge axis -> (core, sup, p, g)):
    s16 = np.ascontiguousarray(
        s.astype(np.float16)
        .reshape(h, ncores, sup, 128, g, m)
        .transpose(1, 2, 3, 0, 4, 5)
        .reshape(ncores, sup, 128, h * g * m)
    )
    v16 = np.ascontiguousarray(
        v.astype(np.float16)
        .reshape(h, ncores, sup, 128, g, m, 3)
        .transpose(1, 2, 3, 0, 6, 4, 5)
        .reshape(ncores, sup, 128, h * 3 * g * m)
    )
    w16 = np.ascontiguousarray(
        w.astype(np.float16).reshape(ncores, sup, 128, g * 4 * h * m)
    )
    r32 = np.ascontiguousarray(r.reshape(ncores, sup, 128, g * 3))

    maps = []
    for c in range(ncores):
        maps.append(
            {
                "s16": s16[c],
                "v16": v16[c],
                "w16": w16[c],
                "r32": r32[c],
                "wpack": wpack,
                "bconst": bconst,
                "ident": ident,
            }
        )
    return maps


def _unarrange(res_list, el, g=G):
    """Convert per-core (sup,128,...) outputs back to reference layout."""
    sup = el // (128 * g)
    so_parts, vo_parts = [], []
    for r in res_list:
        so = (
            r["so"].reshape(sup, 128, H, g, M).transpose(2, 0, 1, 3, 4).reshape(H, el, M)
        )
        vo = (
            r["vo"]
            .reshape(sup, 128, H, g, M, 3)
            .transpose(2, 0, 1, 3, 4, 5)
            .reshape(H, el, M, 3)
        )
        so_parts.append(so)
        vo_parts.append(vo)
    return (
        np.concatenate(so_parts, axis=1),
        np.concatenate(vo_parts, axis=1),
    )


def _get_module():
    if "nc" not in _CACHE:
        _CACHE["nc"] = _build_module(EL, G)
    return _CACHE["nc"]


def kernel(**inputs):
    from concourse.bass_utils import run_bass_kernel_spmd

    nc = _get_module()
    maps = _prep_maps(inputs, EL)
    res = run_bass_kernel_spmd(nc, maps, core_ids=list(range(NCORES)))
    return _unarrange(res.results, EL)
